# revision 7
# baseline (speedup 1.0000x reference)
"""Co-teaching loss (drop-region CE) kernel for Trainium2, 8 NeuronCores.

Reference computation:
  - 2x2 maxpool on inputs1/inputs2 [8,19,512,512] and targets [8,512,512]
  - per-pixel CE loss of each pooled input vs pooled targets -> [8, 65536] x2
  - per-row ascending argsort of each loss map, keep num_remember smallest,
    gather the *other* loss at those indices, return the two scalar means.

Distribution: data-parallel over batch B=8, one batch row per NeuronCore.
Each core computes its row's two pooled CE loss maps [256,256] on-device
(that is all of the memory-bound work: ~41MB of input reads per core).
The tiny top-k selection over the [8, 65536] loss maps (0.5 MB/core out)
is done on host exactly like the reference (stable argsort semantics).

On-device pipeline per core (f32, single pass, no halves):
  - one HWDGE DMA per multi-channel chunk: [p=128, (c, 4rows*512)]; each
    partition p holds raw rows 4p..4p+3 of each chunk channel (8KB
    contiguous per channel per partition).
  - h-pool: one strided TT max per chunk (rows r0v r1, r2 v r3 in one op);
    w-pool: strided TT max -> P[p, c, 2, 256] (pooled rows 2p, 2p+1).
  - targets pooled the same way; masks (tp == c) on GPSIMD.
  - x_t via copy_predicated per channel (DVE); exp chunk in place (ACT);
    channel-sum via strided tensor_reduce (DVE) accumulated into S.
  - loss = Ln(S) - x_t, one 256KB DMA out per input.
"""

import os

import numpy as np

ABLATE = os.environ.get("KERNEL_ABLATE", "")  # "", "dma", "nodma"

B, C, H, W = 8, 19, 512, 512
HP, WP = 256, 256  # pooled spatial dims
L = HP * WP
N_CORES = 8
# channels per DMA/compute chunk (sum = 19)
CHUNKS = [(0, 4), (4, 4), (8, 4), (12, 4), (16, 3)]

_prog_cache = {}

RN = H // 128          # 4 raw rows per partition
RW = RN * W            # 2048 raw elems per channel per partition
PN = RN // 2           # 2 pooled rows per partition
PW = PN * WP           # 512 pooled elems per channel per partition


def _build_program(repeat=1):
    from contextlib import ExitStack

    import concourse.bass as bass  # noqa: F401
    import concourse.mybir as mybir
    import concourse.tile as tile
    from concourse import bacc

    f32 = mybir.dt.float32
    i32 = mybir.dt.int32
    u8 = mybir.dt.uint8
    Alu = mybir.AluOpType
    Act = mybir.ActivationFunctionType

    nc = bacc.Bacc("TRN2", target_bir_lowering=False, debug=False,
                   num_devices=N_CORES)

    x_in = [
        nc.dram_tensor("x1", [C, H, W], f32, kind="ExternalInput"),
        nc.dram_tensor("x2", [C, H, W], f32, kind="ExternalInput"),
    ]
    tg = nc.dram_tensor("tg", [H, W], i32, kind="ExternalInput")
    l_out = [
        nc.dram_tensor("loss1", [HP, WP], f32, kind="ExternalOutput"),
        nc.dram_tensor("loss2", [HP, WP], f32, kind="ExternalOutput"),
    ]

    with tile.TileContext(nc) as tc, ExitStack() as ctx:
        raw_pool = ctx.enter_context(tc.tile_pool(name="raw", bufs=3))
        p_pool = ctx.enter_context(tc.tile_pool(name="pooled", bufs=3))
        tgt_pool = ctx.enter_context(tc.tile_pool(name="tgt", bufs=2))
        mask_pool = ctx.enter_context(tc.tile_pool(name="mask", bufs=2))
        small = ctx.enter_context(tc.tile_pool(name="small", bufs=2))

        for _ in range(repeat):
            # ---- pooled targets: tp [128, 2, 256] int32 ----
            trow = tgt_pool.tile([128, RW], i32, tag="traw")
            nc.sync.dma_start(
                out=trow[:],
                in_=tg.rearrange("h w -> (h w)")
                .rearrange("(p n) -> p n", p=128))
            tv = trow[:].rearrange("p (r w) -> p r w", r=RN)
            # h-pool rows (0,1)->0 and (2,3)->2 in one strided TT
            nc.vector.tensor_tensor(out=tv[:, 0::2, :], in0=tv[:, 0::2, :],
                                    in1=tv[:, 1::2, :], op=Alu.max)
            tp = tgt_pool.tile([128, PW], i32, tag="tp")
            tpv = tp[:].rearrange("p (r w) -> p r w", r=PN)
            nc.vector.tensor_tensor(out=tpv[:], in0=tv[:, 0::2, 0::2],
                                    in1=tv[:, 0::2, 1::2], op=Alu.max)

            # ---- masks (tp == c) for c = 1..18, shared by both inputs ----
            masks = mask_pool.tile([128, (C - 1) * PW], u8, tag="masks")
            for c in range(1, C):
                nc.gpsimd.tensor_scalar(
                    out=masks[:, (c - 1) * PW:c * PW], in0=tp[:],
                    scalar1=float(c), scalar2=None, op0=Alu.is_equal)

            xt12 = small.tile([128, 2 * PW], f32, tag="xt")
            S12 = small.tile([128, 2 * PW], f32, tag="S")
            for (c0, G) in CHUNKS:
                for xi in range(2):
                    xt = xt12[:, xi * PW:(xi + 1) * PW]
                    S = S12[:, xi * PW:(xi + 1) * PW]
                    # one DMA for G channels: [p, (c, rw)]
                    T = raw_pool.tile([128, 4 * RW], f32, tag="T")
                    if ABLATE != "nodma":
                        nc.sync.dma_start(
                            out=T[:, :G * RW]
                            .rearrange("p (c n) -> p c n", c=G),
                            in_=x_in[xi][c0:c0 + G]
                            .rearrange("c (p n) w -> p c (n w)", p=128))
                    else:  # tiny anchor DMA to keep tile deps alive
                        nc.sync.dma_start(
                            out=T[:, :64],
                            in_=x_in[xi][c0, :128, :64])
                    if ABLATE == "dma":
                        continue
                    Tv = T[:, :G * RW].rearrange(
                        "p (c r w) -> p c r w", c=G, r=RN)
                    # h-pool in place (row-parity max), then strided w-pool
                    nc.vector.tensor_tensor(
                        out=Tv[:, :, 0::2, :], in0=Tv[:, :, 0::2, :],
                        in1=Tv[:, :, 1::2, :], op=Alu.max)
                    P = p_pool.tile([128, 4 * PW], f32, tag="P")
                    Pv4 = P[:, :G * PW].rearrange(
                        "p (c r w) -> p c r w", c=G, r=PN)
                    nc.vector.tensor_tensor(
                        out=Pv4[:], in0=Tv[:, :, 0::2, 0::2],
                        in1=Tv[:, :, 0::2, 1::2], op=Alu.max)
                    # x_t updates for this chunk's channels
                    if c0 == 0:
                        nc.vector.tensor_copy(xt, P[:, 0:PW])
                    for c in range(max(c0, 1), c0 + G):
                        nc.vector.copy_predicated(
                            out=xt, mask=masks[:, (c - 1) * PW:c * PW],
                            data=P[:, (c - c0) * PW:(c - c0 + 1) * PW])
                    # exp chunk in place, then channel-sum into S
                    nc.scalar.activation(
                        out=P[:, :G * PW], in_=P[:, :G * PW], func=Act.Exp)
                    if c0 == 0:
                        nc.vector.tensor_reduce(
                            out=S, in_=P[:, :G * PW]
                            .rearrange("p (c q) -> p q c", c=G),
                            axis=mybir.AxisListType.X, op=Alu.add)
                    else:
                        St = p_pool.tile([128, PW], f32, tag="St")
                        nc.vector.tensor_reduce(
                            out=St[:], in_=P[:, :G * PW]
                            .rearrange("p (c q) -> p q c", c=G),
                            axis=mybir.AxisListType.X, op=Alu.add)
                        nc.vector.tensor_add(S, S, St[:])

            # ---- loss = Ln(S) - x_t; one DMA out per input ----
            logS12 = small.tile([128, 2 * PW], f32, tag="logS")
            nc.scalar.activation(out=logS12[:], in_=S12[:], func=Act.Ln)
            lt12 = small.tile([128, 2 * PW], f32, tag="loss")
            nc.vector.tensor_sub(lt12[:], logS12[:], xt12[:])
            for xi in range(2):
                nc.sync.dma_start(
                    out=l_out[xi].rearrange("(p r) w -> p (r w)", r=PN),
                    in_=lt12[:, xi * PW:(xi + 1) * PW])

    nc.compile()
    return nc


def _get_program():
    if "nc" not in _prog_cache:
        _prog_cache["nc"] = _build_program()
    return _prog_cache["nc"]


def _device_loss_maps(inputs1, inputs2, targets):
    """Run the 8-core SPMD kernel; return loss1, loss2 as [8, 65536] f32."""
    from concourse.bass_utils import run_bass_kernel_spmd

    nc = _get_program()
    in_maps = [
        {
            "x1": np.ascontiguousarray(inputs1[b], dtype=np.float32),
            "x2": np.ascontiguousarray(inputs2[b], dtype=np.float32),
            "tg": np.ascontiguousarray(targets[b], dtype=np.int32),
        }
        for b in range(B)
    ]
    res = run_bass_kernel_spmd(nc, in_maps, list(range(N_CORES)))
    loss1 = np.stack([np.asarray(res.results[b]["loss1"]).reshape(L)
                      for b in range(B)])
    loss2 = np.stack([np.asarray(res.results[b]["loss2"]).reshape(L)
                      for b in range(B)])
    return loss1, loss2


def kernel(inputs1, inputs2, targets, forget_rate):
    inputs1 = np.asarray(inputs1, dtype=np.float32)
    inputs2 = np.asarray(inputs2, dtype=np.float32)
    targets = np.asarray(targets, dtype=np.int32)

    loss1, loss2 = _device_loss_maps(inputs1, inputs2, targets)

    num_remember = int((1.0 - float(forget_rate)) * L)
    # stable ascending argsort (matches jnp.argsort) -> keep smallest k,
    # gather the swapped loss, mean.
    ind1 = np.argsort(loss1, axis=1, kind="stable")[:, :num_remember]
    ind2 = np.argsort(loss2, axis=1, kind="stable")[:, :num_remember]
    m1 = np.take_along_axis(loss1, ind2, axis=1).mean(dtype=np.float64)
    m2 = np.take_along_axis(loss2, ind1, axis=1).mean(dtype=np.float64)
    return np.array([m1, m2], dtype=np.float32)
